# revision 51
# baseline (speedup 1.0000x reference)
"""Trainium2 Bass kernel for nn_JointCodebookPredictor.

Strategy: data-parallel over frames (N=16384 -> 2048/core on 8 cores).
Activations kept TRANSPOSED [feature, frame]; matmul contraction dims on
partitions. All big matmuls run fp8(e4m3) with perf_mode=DoubleRow
(2 contraction chunks per pass). Weights are pre-scaled on the host
(W1,LS x32; l2 x8) to dodge fp8 denormals; the scales are undone in the
ReLU evict (scale=1/32) and the exp/target ops (scale=1/8).

Phase 1 (single pass over the 48 feature chunks, full 2048 frames per
PSUM accumulation): hiddenT = relu(W1x32 @ predT + 32*selfpredT)/32+b1,
one-hot operands for the self-regression come pre-built from the host.
BN sum/sumsq accumulate via activation accum_out. hiddenT spilled to
DRAM scratch in fp8. BN stats all-reduce is split in two halves; the
first is issued mid-phase-1 so it overlaps compute.

Phase 2: BN is folded into the small linear2 weights (l2w = s*8*l2) and
a per-(k,c) constant (cst = u @ l2w + 8*b2, u = beta/s - mean), so the
spilled hidden feeds the logits matmul directly with no per-element BN.
logits psum = 8*logits; exp(x/8) with accum gives the softmax denoms;
a one-hot (1/8-valued) dot gives the target-logit sums; a single Ln over
all 256 (k,slot) denominators at the end avoids activation-table churn.
sum(tgt) - sum(lse) -> one scalar per core; the host sums 8 scalars.
"""

import os
import sys

sys.path.insert(0, "/opt/trn_rl_repo")

V_P1ONLY = os.environ.get("V_P1ONLY", "0") == "1"  # debug: skip phase 2

import numpy as np
import ml_dtypes

from concourse import bass, tile, bacc
import concourse.mybir as mybir
from concourse.bass_utils import run_bass_kernel_spmd

BF16 = mybir.dt.bfloat16
F32 = mybir.dt.float32
F8 = mybir.dt.float8e4

NCB = 16
CS = 256
HD = 384
PD = 2048
N_TOT = 16384
NCORES = 8
NF = N_TOT // NCORES          # 2048 frames per core
FCH = (NCB * HD) // 128       # 48 feature chunks of 128
KC = PD // 128                # 16 contraction chunks for mm1
BN_EPS = 1e-5
SELF_F0 = HD // 128           # first self feature chunk (=3)
MAXSLOT = (NCB - 1) * 2       # 30 slots in padded LS layout
FH1 = FCH // 2                # stats half boundary (f 0..23 = codebooks 0..7)
W1S = 32.0                    # host pre-scale on W1/LS
L2S = 8.0                     # host pre-scale on linear2/bias2

_cached = {}


def _build_program():
    nc = bacc.Bacc("TRN2", target_bir_lowering=False, debug=False,
                   num_devices=NCORES)

    # ---- DRAM I/O -------------------------------------------------------
    predT_d = nc.dram_tensor("predT", [128, KC, NF], F8, kind="ExternalInput")
    w1t_d = nc.dram_tensor("w1t", [FCH, 128, KC, 128], F8, kind="ExternalInput")
    lst_d = nc.dram_tensor("lst", [FCH - SELF_F0, 128, MAXSLOT, 128], F8,
                           kind="ExternalInput")
    ohT_d = nc.dram_tensor("ohT", [128, NCB, 2, NF], F8, kind="ExternalInput")
    ohq_d = nc.dram_tensor("ohq", [NCB, 128, NF // 128, CS], F8,
                           kind="ExternalInput")
    l2t_d = nc.dram_tensor("l2t", [NCB, 128, 3, CS], F8, kind="ExternalInput")
    b2t_d = nc.dram_tensor("b2t", [1, NCB, CS], F32, kind="ExternalInput")
    b1_d = nc.dram_tensor("b1t", [128, FCH], F32, kind="ExternalInput")
    g_d = nc.dram_tensor("gt", [128, FCH], F32, kind="ExternalInput")
    be_d = nc.dram_tensor("bet", [128, FCH], F32, kind="ExternalInput")
    ones_d = nc.dram_tensor("onesF", [128, 1], F32, kind="ExternalInput")
    oneck_d = nc.dram_tensor("oneck", [128, 512], F8, kind="ExternalInput")

    lp_d = nc.dram_tensor("lp", [1, 1], F32, kind="ExternalOutput")
    dbg_d = nc.dram_tensor("dbg", [128, 2 * FCH], F32, kind="ExternalOutput")

    AR = mybir.AluOpType
    AF = mybir.ActivationFunctionType
    DR = mybir.MatmulPerfMode.DoubleRow
    from contextlib import ExitStack

    with tile.TileContext(nc) as tc:
        with ExitStack() as ctx:
            ep = ctx.enter_context
            constp = ep(tc.tile_pool(name="const", bufs=1))
            predp = ep(tc.tile_pool(name="pred", bufs=1))
            ohp = ep(tc.tile_pool(name="oh", bufs=1))
            w1p = ep(tc.tile_pool(name="w1", bufs=2))
            lsp = ep(tc.tile_pool(name="ls", bufs=2))
            htp = ep(tc.tile_pool(name="ht", bufs=3))
            sqp = ep(tc.tile_pool(name="sq", bufs=2))
            statp = ep(tc.tile_pool(name="stat", bufs=1))
            l2wp = ep(tc.tile_pool(name="l2w", bufs=1))
            l2rp = ep(tc.tile_pool(name="l2r", bufs=3))
            ohqp = ep(tc.tile_pool(name="ohq", bufs=3))
            hlp = ep(tc.tile_pool(name="hload", bufs=6))
            expp = ep(tc.tile_pool(name="expp", bufs=6))
            dotp = ep(tc.tile_pool(name="dotp", bufs=2))
            accp = ep(tc.tile_pool(name="acc", bufs=1))
            dramp = ep(tc.tile_pool(name="dram", bufs=1, space="DRAM"))

            # ---- resident constants ---------------------------------
            b1sb = constp.tile([128, FCH], F32)
            nc.sync.dma_start(b1sb[:], b1_d[:])
            gsb = constp.tile([128, FCH], F32)
            nc.sync.dma_start(gsb[:], g_d[:])
            besb = constp.tile([128, FCH], F32)
            nc.sync.dma_start(besb[:], be_d[:])
            b2sb = constp.tile([1, NCB, CS], F32)
            nc.sync.dma_start(b2sb[:], b2t_d[:])
            onesF = constp.tile([128, 1], F32)
            nc.sync.dma_start(onesF[:], ones_d[:])

            # predT split per contraction-pair so the first matmul only
            # waits for 1/8 of the 4MB; first W1 slice loads before the
            # predT bulk; ohT loads are issued inside the f-loop (ohT[k]
            # first used at f=3+3k)
            predT = predp.tile([128, KC, NF], F8)
            w1sl0 = w1p.tile([128, KC, 128], F8, tag="w1sl")
            nc.sync.dma_start(predT[:, 0:2, :], predT_d[:, 0:2, :])
            nc.sync.dma_start(w1sl0[:], w1t_d[0][:])
            for c in range(1, KC // 2):
                nc.sync.dma_start(predT[:, 2 * c:2 * c + 2, :],
                                  predT_d[:, 2 * c:2 * c + 2, :])
            ohT = ohp.tile([128, NCB, 2, NF], F8)

            scratch = dramp.tile([FCH, 128, NF], F8)
            st_in1 = dramp.tile([128, 2 * FH1], F32)
            st_out1 = dramp.tile([128, 2 * FH1], F32)
            st_in2 = dramp.tile([128, 2 * FH1], F32)
            st_out2 = dramp.tile([128, 2 * FH1], F32)

            ssum = statp.tile([128, FCH], F32)
            ssq = statp.tile([128, FCH], F32)
            pack1 = statp.tile([128, 2 * FH1], F32)
            pack2 = statp.tile([128, 2 * FH1], F32)

            # ================= phase 1 ===============================
            with tc.tile_pool(name="ps1", bufs=2,
                              space=bass.MemorySpace.PSUM) as ps1:
                for f in range(FCH):
                    if f == 0:
                        w1sl = w1sl0
                    else:
                        w1sl = w1p.tile([128, KC, 128], F8, tag="w1sl")
                        nc.sync.dma_start(w1sl[:], w1t_d[f][:])
                    if f % 3 == 0 and f // 3 < NCB - 1:
                        kk = f // 3
                        nc.sync.dma_start(ohT[:, kk, :, :],
                                          ohT_d[:, kk, :, :])
                    i_blk = -1
                    if f >= SELF_F0:
                        i_blk = (f - SELF_F0) // 3
                        nslot = (i_blk + 1) * 2
                        lssl = lsp.tile([128, MAXSLOT, 128], F8, tag="lssl")
                        nc.sync.dma_start(lssl[:, 0:nslot, :],
                                          lst_d[f - SELF_F0][:, 0:nslot, :])
                    pt = ps1.tile([128, NF], F32, tag="p1")
                    for c in range(KC // 2):
                        for h in range(4):
                            hs = slice(h * 512, (h + 1) * 512)
                            last = (c == KC // 2 - 1) and i_blk < 0
                            nc.tensor.matmul(
                                pt[:, hs],
                                w1sl[:, 2 * c:2 * c + 2, :],
                                predT[:, 2 * c:2 * c + 2, hs],
                                start=(c == 0), stop=last,
                                perf_mode=DR, skip_group_check=True)
                    if i_blk >= 0:
                        for k in range(i_blk + 1):
                            for h in range(4):
                                hs = slice(h * 512, (h + 1) * 512)
                                nc.tensor.matmul(
                                    pt[:, hs],
                                    lssl[:, 2 * k:2 * k + 2, :],
                                    ohT[:, k, :, hs],
                                    start=False, stop=(k == i_blk),
                                    perf_mode=DR, skip_group_check=True)
                    # evict: relu(x/32 + b1), accumulate BN sum / sumsq
                    ht = htp.tile([128, NF], F8, tag="ht")
                    nc.scalar.activation(
                        ht[:], pt[:], AF.Relu,
                        bias=b1sb[:, f:f + 1], scale=1.0 / W1S,
                        accum_out=ssum[:, f:f + 1])
                    sq = sqp.tile([128, NF], BF16, tag="sq")
                    nc.vector.tensor_tensor(sq[:], ht[:], ht[:], AR.mult)
                    nc.vector.tensor_reduce(ssq[:, f:f + 1], sq[:],
                                            mybir.AxisListType.X, AR.add)
                    nc.sync.dma_start(scratch[f][:], ht[:])

                    if f == FH1 - 1:
                        # first-half stats allreduce, overlapped with
                        # the remaining phase-1 compute
                        nc.vector.tensor_copy(pack1[:, 0:FH1],
                                              ssum[:, 0:FH1])
                        nc.vector.tensor_copy(pack1[:, FH1:], ssq[:, 0:FH1])
                        nc.sync.dma_start(st_in1[:], pack1[:])
                        nc.gpsimd.collective_compute(
                            "AllReduce", AR.add,
                            replica_groups=[list(range(NCORES))],
                            ins=[st_in1.opt()], outs=[st_out1.opt()])


                nc.vector.tensor_copy(pack2[:, 0:FH1], ssum[:, FH1:])
                nc.vector.tensor_copy(pack2[:, FH1:], ssq[:, FH1:])
                nc.sync.dma_start(st_in2[:], pack2[:])
                nc.gpsimd.collective_compute(
                    "AllReduce", AR.add,
                    replica_groups=[list(range(NCORES))],
                    ins=[st_in2.opt()], outs=[st_out2.opt()])

            # ============ BN finalize (per half) =====================
            allst1 = statp.tile([128, 2 * FH1], F32)
            allst2 = statp.tile([128, 2 * FH1], F32)
            sTw = statp.tile([128, FCH], F32)    # gamma / std
            u_bf = statp.tile([128, FCH], F8)  # beta/sTw - mean

            def bn_finalize(allst, st_out, hs):
                nc.sync.dma_start(allst[:], st_out[:])
                mean = statp.tile([128, FH1], F32, tag="mean")
                nc.vector.tensor_scalar(mean[:], allst[:, 0:FH1],
                                        1.0 / N_TOT, None, AR.mult)
                vpe = statp.tile([128, FH1], F32, tag="vpe")
                nc.vector.tensor_scalar(vpe[:], allst[:, FH1:],
                                        1.0 / N_TOT, None, AR.mult)
                m2 = statp.tile([128, FH1], F32, tag="m2")
                nc.vector.tensor_tensor(m2[:], mean[:], mean[:], AR.mult)
                nc.vector.tensor_tensor(vpe[:], vpe[:], m2[:], AR.subtract)
                nc.vector.tensor_scalar(vpe[:], vpe[:], BN_EPS, None, AR.add)
                sqv = statp.tile([128, FH1], F32, tag="sqv")
                nc.scalar.activation(sqv[:], vpe[:], AF.Sqrt)
                rr = statp.tile([128, FH1], F32, tag="rr")
                nc.vector.reciprocal(rr[:], sqv[:])
                t1 = statp.tile([128, FH1], F32, tag="t1")
                for _ in range(2):
                    nc.vector.tensor_tensor(t1[:], rr[:], rr[:], AR.mult)
                    nc.vector.tensor_tensor(t1[:], t1[:], vpe[:], AR.mult)
                    nc.vector.tensor_scalar(t1[:], t1[:], -0.5, 1.5,
                                            AR.mult, AR.add)
                    nc.vector.tensor_tensor(rr[:], rr[:], t1[:], AR.mult)
                nc.vector.tensor_tensor(sTw[:, hs], rr[:], gsb[:, hs],
                                        AR.mult)
                # u = beta / sTw - mean
                rs = statp.tile([128, FH1], F32, tag="rs")
                nc.vector.reciprocal(rs[:], sTw[:, hs])
                uf = statp.tile([128, FH1], F32, tag="uf")
                nc.vector.tensor_tensor(uf[:], besb[:, hs], rs[:], AR.mult)
                nc.vector.tensor_tensor(uf[:], uf[:], mean[:], AR.subtract)
                nc.scalar.activation(u_bf[:, hs], uf[:], AF.Copy)

            bn_finalize(allst1, st_out1, slice(0, FH1))
            nc.sync.dma_start(dbg_d[:, 0:FH1], allst1[:, 0:FH1])
            nc.sync.dma_start(dbg_d[:, FCH:FCH + FH1], allst1[:, FH1:])

            # ================= phase 2 ===============================
            if V_P1ONLY:
                bn_finalize(allst2, st_out2, slice(FH1, FCH))
                nc.sync.dma_start(dbg_d[:, FH1:FCH], allst2[:, 0:FH1])
                nc.sync.dma_start(dbg_d[:, FCH + FH1:], allst2[:, FH1:])
                outsb0 = accp.tile([1, 1], F32)
                nc.vector.tensor_copy(outsb0[:], sTw[0:1, 0:1])
                nc.sync.dma_start(lp_d[:], outsb0[:])
            # l2w chunk 3 is the bias row: partition 0 holds cst8[k,:],
            # partitions 1..127 are zero; paired with hl chunk 3 whose
            # partition 0 is all-ones. This folds the per-(k,c) constant
            # into the second DoubleRow matmul - no separate bias matmul.
            l2w = l2wp.tile([128, NCB, 4, CS], F8)
            cstf = constp.tile([1, CS], F32)
            NSL = NF // 128  # 16 frame slots of 128
            ekall = statp.tile([128, NCB, NSL], F32)
            tgtall = statp.tile([128, NCB, 4], F32)

            with tc.tile_pool(name="ps2", bufs=3,
                              space=bass.MemorySpace.PSUM) as ps2, \
                 tc.tile_pool(name="psC", bufs=1,
                              space=bass.MemorySpace.PSUM) as psC:
                for k in range(NCB):
                    nc.vector.memset(l2w[:, k, 3, :], 0.0)
                def prep_k(k):
                    # fold BN scale into l2 weights; build cst row
                    l2r = l2rp.tile([128, 3, CS], F8, tag="l2r")
                    nc.sync.dma_start(l2r[:], l2t_d[k][:])
                    for c in range(3):
                        f = 3 * k + c
                        nc.scalar.activation(l2w[:, k, c, :], l2r[:, c, :],
                                             AF.Copy, scale=sTw[:, f:f + 1])
                    pc = psC.tile([1, CS], F32, tag="pc")
                    for c in range(3):
                        f = 3 * k + c
                        nc.tensor.matmul(pc[:], u_bf[:, f:f + 1],
                                         l2w[:, k, c, :],
                                         start=(c == 0), stop=(c == 2),
                                         skip_group_check=True)
                    nc.vector.tensor_tensor(cstf[:], pc[:],
                                            b2sb[0:1, k, :], AR.add)
                    nc.scalar.activation(l2w[0:1, k, 3, :], cstf[:], AF.Copy)

                def slots_k(k):
                    ohqk = ohqp.tile([128, NSL, CS], F8, tag="ohqk")
                    nc.sync.dma_start(ohqk[:], ohq_d[k][:])
                    # prefetch all 4 j2 hidden blocks up front
                    hls = []
                    for j2 in range(4):
                        hl = hlp.tile([128, 4, 512], F8, tag="hl")
                        for c in range(3):
                            nc.sync.dma_start(
                                hl[:, c, :],
                                scratch[3 * k + c][:, j2 * 512:(j2 + 1) * 512])
                        # chunk 3 = ones row (partition 0) for the folded
                        # bias; pairs with the cst row in l2w chunk 3
                        nc.sync.dma_start(hl[:, 3, :], oneck_d[:])
                        hls.append(hl)
                    for j2 in range(4):
                        hl = hls[j2]
                        pt2 = ps2.tile([128, 4, CS], F32, tag="p2")
                        # weight-grouped order: 4x pair1 then 4x pair2.
                        # start=True only on the first 1KB region of each
                        # 2KB PSUM zero-region (bank): the start marks the
                        # whole bank pending-zero, so the sibling region's
                        # first write (start=False) still zero-initializes;
                        # a second start=True would wipe the sibling's sum.
                        for q in range(4):
                            nc.tensor.matmul(
                                pt2[:, q, :],
                                hl[:, 0:2, q * 128:(q + 1) * 128],
                                l2w[:, k, 0:2, :],
                                start=(q % 2 == 0), stop=False,
                                perf_mode=DR, skip_group_check=True)
                        for q in range(4):
                            nc.tensor.matmul(
                                pt2[:, q, :],
                                hl[:, 2:4, q * 128:(q + 1) * 128],
                                l2w[:, k, 2:4, :],
                                start=False, stop=True,
                                perf_mode=DR, skip_group_check=True)
                        for q in range(4):
                            sl = j2 * 4 + q
                            ex = expp.tile([128, CS], BF16, tag="ex")
                            nc.scalar.activation(
                                ex[:], pt2[:, q, :], AF.Exp,
                                bias=0.0, scale=1.0 / L2S,
                                accum_out=ekall[:, k, sl:sl + 1])
                        # ohq holds 1/8 (not 1.0) -> descales the x8
                        # weight prescale in the target dot for free
                        dot = dotp.tile([128, 4, CS], F32, tag="dot")
                        nc.vector.tensor_tensor(
                            dot[:], pt2[:],
                            ohqk[:, 4 * j2:4 * j2 + 4, :], AR.mult)
                        nc.vector.tensor_reduce(
                            tgtall[:, k, j2:j2 + 1], dot[:],
                            mybir.AxisListType.XY, AR.add)

                if not V_P1ONLY:
                    for k in range(NCB // 2):
                        prep_k(k)
                    slots_k(0)
                    slots_k(1)
                    bn_finalize(allst2, st_out2, slice(FH1, FCH))
                    nc.sync.dma_start(dbg_d[:, FH1:FCH], allst2[:, 0:FH1])
                    nc.sync.dma_start(dbg_d[:, FCH + FH1:], allst2[:, FH1:])
                    for k in range(NCB // 2, NCB):
                        prep_k(k)
                    for k in range(2, NCB):
                        slots_k(k)

                if not V_P1ONLY:
                    # sum(tgt - lse) = sum(tgt) - sum(ln(esum)); one Ln
                    # instruction for all 256 (k, slot) esums
                    lseall = statp.tile([128, NCB, NSL], F32)
                    nc.scalar.activation(lseall[:], ekall[:], AF.Ln)
                    lsesum = accp.tile([128, 1], F32)
                    nc.vector.tensor_reduce(lsesum[:], lseall[:],
                                            mybir.AxisListType.XY, AR.add)
                    tgtsum = accp.tile([128, 1], F32)
                    nc.vector.tensor_reduce(tgtsum[:], tgtall[:],
                                            mybir.AxisListType.XY, AR.add)
                    total = accp.tile([128, 1], F32)
                    nc.vector.tensor_tensor(total[:], tgtsum[:], lsesum[:],
                                            AR.subtract)
                    ptF = psC.tile([1, 1], F32, tag="ptF")
                    nc.tensor.matmul(ptF[:], onesF[:], total[:], start=True,
                                     stop=True)
                    outsb = accp.tile([1, 1], F32)
                    nc.vector.tensor_copy(outsb[:], ptF[:])
                    nc.sync.dma_start(lp_d[:], outsb[:])

    nc.compile()
    return nc


def _prep_inputs(predictor, codebook_indexes, W1, b1, linear_self,
                 bn_gamma, bn_beta, linear2, bias2):
    f8 = ml_dtypes.float8_e4m3
    one8 = np.array(1.0, dtype=f8).view(np.uint8)
    ci = np.asarray(codebook_indexes).astype(np.int32)
    pred = np.asarray(predictor, dtype=np.float32)

    # replicated weights
    # W1 tiled: [f, kr, kc, m] = 32*W1[f*128+m, kc*128+kr]
    w1t = np.ascontiguousarray(
        (W1 * W1S).reshape(FCH, 128, KC, 128).transpose(0, 3, 2, 1)).astype(f8)
    # masked-by-construction LS: [f', cr, slot=k*2+cc, m] =
    #   32*linear_self[f'*128+m, k*256+cc*128+cr]
    ls = np.asarray(linear_self, dtype=np.float32) * W1S
    ls4 = ls.reshape(FCH - SELF_F0, 128, NCB - 1, 2, 128)  # [f', m, k, cc, cr]
    lst = np.zeros((FCH - SELF_F0, 128, MAXSLOT, 128), dtype=f8)
    lst_full = ls4.transpose(0, 4, 2, 3, 1).reshape(FCH - SELF_F0, 128,
                                                    MAXSLOT, 128)
    for fp in range(FCH - SELF_F0):
        nslot = ((fp // 3) + 1) * 2
        lst[fp, :, 0:nslot, :] = lst_full[fp, :, 0:nslot, :].astype(f8)
    # l2 tiled: [k, hr, hc, c] = 8*linear2[k, c, hc*128+hr]
    l2t = np.ascontiguousarray(
        (np.asarray(linear2, dtype=np.float32) * L2S)
        .reshape(NCB, CS, 3, 128).transpose(0, 3, 2, 1)).astype(f8)
    b2t = (np.asarray(bias2, dtype=np.float32) * L2S).reshape(1, NCB, CS)
    b1t = np.ascontiguousarray(
        np.asarray(b1, dtype=np.float32).reshape(FCH, 128).T)
    gt = np.ascontiguousarray(
        np.asarray(bn_gamma, dtype=np.float32).reshape(FCH, 128).T)
    bet = np.ascontiguousarray(
        np.asarray(bn_beta, dtype=np.float32).reshape(FCH, 128).T)
    onesF = np.ones((128, 1), dtype=np.float32)
    oneck = np.zeros((128, 512), dtype=f8)
    oneck[0, :] = 1.0

    rep = dict(w1t=w1t, lst=lst, l2t=l2t, b2t=b2t, b1t=b1t, gt=gt, bet=bet,
               onesF=onesF, oneck=oneck)

    NSL = NF // 128
    nn = np.arange(NF)
    kk = np.broadcast_to(np.arange(NCB), (NF, NCB))
    nn2 = np.broadcast_to(nn[:, None], (NF, NCB))
    in_maps = []
    for cidx in range(NCORES):
        sl = slice(cidx * NF, (cidx + 1) * NF)
        # predT tiled: [kr, kc, n] = pred[n, kc*128+kr]
        predT = np.ascontiguousarray(
            pred[sl].reshape(NF, KC, 128).transpose(2, 1, 0)).astype(f8)
        cil = ci[sl]                                   # (NF, NCB)
        valid = (cil >= 0) & (cil < CS)
        cc = np.clip(cil, 0, CS - 1)
        # ohT[r, k, hi, n] = 1 iff idx[n,k] == hi*128+r
        ohT = np.zeros((128, NCB, 2, NF), dtype=np.uint8)
        ohT[(cc % 128)[valid], kk[valid], (cc // 128)[valid],
            nn2[valid]] = one8
        # ohq[k, p, slot=n//128, c] = 1/8 iff idx[n,k] == c  (fp8);
        # the 1/8 undoes the x8 linear2 prescale in the target dot
        eighth = np.array(0.125, dtype=f8).view(np.uint8)
        ohq = np.zeros((NCB, 128, NSL, CS), dtype=np.uint8)
        ohq[kk[valid], (nn2 % 128)[valid], (nn2 // 128)[valid],
            cc[valid]] = eighth
        m = dict(predT=predT, ohT=ohT.view(f8), ohq=ohq.view(f8))
        m.update(rep)
        in_maps.append(m)
    return in_maps


def kernel(predictor, codebook_indexes, W1, b1, linear_self,
           bn_gamma, bn_beta, linear2, bias2):
    if "nc" not in _cached:
        _cached["nc"] = _build_program()
    nc = _cached["nc"]
    in_maps = _prep_inputs(predictor, codebook_indexes, W1, b1, linear_self,
                           bn_gamma, bn_beta, linear2, bias2)
    res = run_bass_kernel_spmd(nc, in_maps, list(range(NCORES)))
    _cached["last_results"] = res
    tot_logprob = np.float32(sum(float(r["lp"][0, 0]) for r in res.results))
    ci = np.asarray(codebook_indexes)
    tot_count = np.int32((ci[:, 0] >= 0).sum())
    return tot_logprob, tot_count


# revision 52
# speedup vs baseline: 1.0197x; 1.0197x over previous
"""Trainium2 Bass kernel for nn_JointCodebookPredictor.

Strategy: data-parallel over frames (N=16384 -> 2048/core on 8 cores).
Activations kept TRANSPOSED [feature, frame]; matmul contraction dims on
partitions. All big matmuls run fp8(e4m3) with perf_mode=DoubleRow
(2 contraction chunks per pass). Weights are pre-scaled on the host
(W1,LS x32; l2 x8) to dodge fp8 denormals; the scales are undone in the
ReLU evict (scale=1/32) and the exp/target ops (scale=1/8).

Phase 1 (single pass over the 48 feature chunks, full 2048 frames per
PSUM accumulation): hiddenT = relu(W1x32 @ predT + 32*selfpredT)/32+b1,
one-hot operands for the self-regression come pre-built from the host.
BN sum/sumsq accumulate via activation accum_out. hiddenT spilled to
DRAM scratch in fp8. BN stats all-reduce is split in two halves; the
first is issued mid-phase-1 so it overlaps compute.

Phase 2: BN is folded into the small linear2 weights (l2w = s*8*l2) and
a per-(k,c) constant (cst = u @ l2w + 8*b2, u = beta/s - mean), so the
spilled hidden feeds the logits matmul directly with no per-element BN.
logits psum = 8*logits; exp(x/8) with accum gives the softmax denoms;
a one-hot (1/8-valued) dot gives the target-logit sums; a single Ln over
all 256 (k,slot) denominators at the end avoids activation-table churn.
sum(tgt) - sum(lse) -> one scalar per core; the host sums 8 scalars.
"""

import os
import sys

sys.path.insert(0, "/opt/trn_rl_repo")

V_P1ONLY = os.environ.get("V_P1ONLY", "0") == "1"  # debug: skip phase 2

import numpy as np
import ml_dtypes

from concourse import bass, tile, bacc
import concourse.mybir as mybir
from concourse.bass_utils import run_bass_kernel_spmd

BF16 = mybir.dt.bfloat16
F32 = mybir.dt.float32
F8 = mybir.dt.float8e4

NCB = 16
CS = 256
HD = 384
PD = 2048
N_TOT = 16384
NCORES = 8
NF = N_TOT // NCORES          # 2048 frames per core
FCH = (NCB * HD) // 128       # 48 feature chunks of 128
KC = PD // 128                # 16 contraction chunks for mm1
BN_EPS = 1e-5
SELF_F0 = HD // 128           # first self feature chunk (=3)
MAXSLOT = (NCB - 1) * 2       # 30 slots in padded LS layout
FH1 = FCH // 2                # stats half boundary (f 0..23 = codebooks 0..7)
W1S = 32.0                    # host pre-scale on W1/LS
L2S = 8.0                     # host pre-scale on linear2/bias2

_cached = {}


def _build_program():
    nc = bacc.Bacc("TRN2", target_bir_lowering=False, debug=False,
                   num_devices=NCORES)

    # ---- DRAM I/O -------------------------------------------------------
    predT_d = nc.dram_tensor("predT", [128, KC, NF], F8, kind="ExternalInput")
    w1t_d = nc.dram_tensor("w1t", [FCH, 128, KC, 128], F8, kind="ExternalInput")
    lst_d = nc.dram_tensor("lst", [FCH - SELF_F0, 128, MAXSLOT, 128], F8,
                           kind="ExternalInput")
    ohT_d = nc.dram_tensor("ohT", [128, NCB, 2, NF], F8, kind="ExternalInput")
    ohq_d = nc.dram_tensor("ohq", [NCB, 128, NF // 128, CS], F8,
                           kind="ExternalInput")
    l2t_d = nc.dram_tensor("l2t", [NCB, 128, 3, CS], F8, kind="ExternalInput")
    b2t_d = nc.dram_tensor("b2t", [1, NCB, CS], F32, kind="ExternalInput")
    b1_d = nc.dram_tensor("b1t", [128, FCH], F32, kind="ExternalInput")
    g_d = nc.dram_tensor("gt", [128, FCH], F32, kind="ExternalInput")
    be_d = nc.dram_tensor("bet", [128, FCH], F32, kind="ExternalInput")
    ones_d = nc.dram_tensor("onesF", [128, 1], F32, kind="ExternalInput")
    oneck_d = nc.dram_tensor("oneck", [128, 512], F8, kind="ExternalInput")

    lp_d = nc.dram_tensor("lp", [1, 1], F32, kind="ExternalOutput")
    dbg_d = nc.dram_tensor("dbg", [128, 2 * FCH], F32, kind="ExternalOutput")

    AR = mybir.AluOpType
    AF = mybir.ActivationFunctionType
    DR = mybir.MatmulPerfMode.DoubleRow
    from contextlib import ExitStack

    with tile.TileContext(nc) as tc:
        with ExitStack() as ctx:
            ep = ctx.enter_context
            constp = ep(tc.tile_pool(name="const", bufs=1))
            predp = ep(tc.tile_pool(name="pred", bufs=1))
            ohp = ep(tc.tile_pool(name="oh", bufs=1))
            w1p = ep(tc.tile_pool(name="w1", bufs=2))
            lsp = ep(tc.tile_pool(name="ls", bufs=2))
            htp = ep(tc.tile_pool(name="ht", bufs=3))
            sqp = ep(tc.tile_pool(name="sq", bufs=2))
            statp = ep(tc.tile_pool(name="stat", bufs=1))
            l2wp = ep(tc.tile_pool(name="l2w", bufs=1))
            l2rp = ep(tc.tile_pool(name="l2r", bufs=3))
            ohqp = ep(tc.tile_pool(name="ohq", bufs=3))
            hlp = ep(tc.tile_pool(name="hload", bufs=6))
            expp = ep(tc.tile_pool(name="expp", bufs=6))
            dotp = ep(tc.tile_pool(name="dotp", bufs=3))
            accp = ep(tc.tile_pool(name="acc", bufs=1))
            dramp = ep(tc.tile_pool(name="dram", bufs=1, space="DRAM"))

            # ---- resident constants ---------------------------------
            b1sb = constp.tile([128, FCH], F32)
            nc.sync.dma_start(b1sb[:], b1_d[:])
            gsb = constp.tile([128, FCH], F32)
            nc.sync.dma_start(gsb[:], g_d[:])
            besb = constp.tile([128, FCH], F32)
            nc.sync.dma_start(besb[:], be_d[:])
            b2sb = constp.tile([1, NCB, CS], F32)
            nc.sync.dma_start(b2sb[:], b2t_d[:])
            onesF = constp.tile([128, 1], F32)
            nc.sync.dma_start(onesF[:], ones_d[:])

            # predT split per contraction-pair so the first matmul only
            # waits for 1/8 of the 4MB; first W1 slice loads before the
            # predT bulk; ohT loads are issued inside the f-loop (ohT[k]
            # first used at f=3+3k)
            predT = predp.tile([128, KC, NF], F8)
            w1sl0 = w1p.tile([128, KC, 128], F8, tag="w1sl")
            nc.sync.dma_start(predT[:, 0:2, :], predT_d[:, 0:2, :])
            nc.sync.dma_start(w1sl0[:], w1t_d[0][:])
            for c in range(1, KC // 2):
                nc.sync.dma_start(predT[:, 2 * c:2 * c + 2, :],
                                  predT_d[:, 2 * c:2 * c + 2, :])
            ohT = ohp.tile([128, NCB, 2, NF], F8)

            scratch = dramp.tile([FCH, 128, NF], F8)
            st_in1 = dramp.tile([128, 2 * FH1], F32)
            st_out1 = dramp.tile([128, 2 * FH1], F32)
            st_in2 = dramp.tile([128, 2 * FH1], F32)
            st_out2 = dramp.tile([128, 2 * FH1], F32)

            ssum = statp.tile([128, FCH], F32)
            ssq = statp.tile([128, FCH], F32)
            pack1 = statp.tile([128, 2 * FH1], F32)
            pack2 = statp.tile([128, 2 * FH1], F32)

            # ================= phase 1 ===============================
            with tc.tile_pool(name="ps1", bufs=2,
                              space=bass.MemorySpace.PSUM) as ps1:
                for f in range(FCH):
                    if f == 0:
                        w1sl = w1sl0
                    else:
                        w1sl = w1p.tile([128, KC, 128], F8, tag="w1sl")
                        nc.sync.dma_start(w1sl[:], w1t_d[f][:])
                    if f % 3 == 0 and f // 3 < NCB - 1:
                        kk = f // 3
                        nc.sync.dma_start(ohT[:, kk, :, :],
                                          ohT_d[:, kk, :, :])
                    i_blk = -1
                    if f >= SELF_F0:
                        i_blk = (f - SELF_F0) // 3
                        nslot = (i_blk + 1) * 2
                        lssl = lsp.tile([128, MAXSLOT, 128], F8, tag="lssl")
                        nc.sync.dma_start(lssl[:, 0:nslot, :],
                                          lst_d[f - SELF_F0][:, 0:nslot, :])
                    pt = ps1.tile([128, NF], F32, tag="p1")
                    for c in range(KC // 2):
                        for h in range(4):
                            hs = slice(h * 512, (h + 1) * 512)
                            last = (c == KC // 2 - 1) and i_blk < 0
                            nc.tensor.matmul(
                                pt[:, hs],
                                w1sl[:, 2 * c:2 * c + 2, :],
                                predT[:, 2 * c:2 * c + 2, hs],
                                start=(c == 0), stop=last,
                                perf_mode=DR, skip_group_check=True)
                    if i_blk >= 0:
                        for k in range(i_blk + 1):
                            for h in range(4):
                                hs = slice(h * 512, (h + 1) * 512)
                                nc.tensor.matmul(
                                    pt[:, hs],
                                    lssl[:, 2 * k:2 * k + 2, :],
                                    ohT[:, k, :, hs],
                                    start=False, stop=(k == i_blk),
                                    perf_mode=DR, skip_group_check=True)
                    # evict: relu(x/32 + b1), accumulate BN sum / sumsq
                    ht = htp.tile([128, NF], F8, tag="ht")
                    nc.scalar.activation(
                        ht[:], pt[:], AF.Relu,
                        bias=b1sb[:, f:f + 1], scale=1.0 / W1S,
                        accum_out=ssum[:, f:f + 1])
                    sq = sqp.tile([128, NF], BF16, tag="sq")
                    nc.vector.tensor_tensor(sq[:], ht[:], ht[:], AR.mult)
                    nc.vector.tensor_reduce(ssq[:, f:f + 1], sq[:],
                                            mybir.AxisListType.X, AR.add)
                    nc.sync.dma_start(scratch[f][:], ht[:])

                    if f == FH1 - 1:
                        # first-half stats allreduce, overlapped with
                        # the remaining phase-1 compute
                        nc.vector.tensor_copy(pack1[:, 0:FH1],
                                              ssum[:, 0:FH1])
                        nc.vector.tensor_copy(pack1[:, FH1:], ssq[:, 0:FH1])
                        nc.sync.dma_start(st_in1[:], pack1[:])
                        nc.gpsimd.collective_compute(
                            "AllReduce", AR.add,
                            replica_groups=[list(range(NCORES))],
                            ins=[st_in1.opt()], outs=[st_out1.opt()])


                nc.vector.tensor_copy(pack2[:, 0:FH1], ssum[:, FH1:])
                nc.vector.tensor_copy(pack2[:, FH1:], ssq[:, FH1:])
                nc.sync.dma_start(st_in2[:], pack2[:])
                nc.gpsimd.collective_compute(
                    "AllReduce", AR.add,
                    replica_groups=[list(range(NCORES))],
                    ins=[st_in2.opt()], outs=[st_out2.opt()])

            # ============ BN finalize (per half) =====================
            allst1 = statp.tile([128, 2 * FH1], F32)
            allst2 = statp.tile([128, 2 * FH1], F32)
            sTw = statp.tile([128, FCH], F32)    # gamma / std
            u_bf = statp.tile([128, FCH], F8)  # beta/sTw - mean

            def bn_finalize(allst, st_out, hs):
                nc.sync.dma_start(allst[:], st_out[:])
                mean = statp.tile([128, FH1], F32, tag="mean")
                nc.vector.tensor_scalar(mean[:], allst[:, 0:FH1],
                                        1.0 / N_TOT, None, AR.mult)
                vpe = statp.tile([128, FH1], F32, tag="vpe")
                nc.vector.tensor_scalar(vpe[:], allst[:, FH1:],
                                        1.0 / N_TOT, None, AR.mult)
                m2 = statp.tile([128, FH1], F32, tag="m2")
                nc.vector.tensor_tensor(m2[:], mean[:], mean[:], AR.mult)
                nc.vector.tensor_tensor(vpe[:], vpe[:], m2[:], AR.subtract)
                nc.vector.tensor_scalar(vpe[:], vpe[:], BN_EPS, None, AR.add)
                sqv = statp.tile([128, FH1], F32, tag="sqv")
                nc.scalar.activation(sqv[:], vpe[:], AF.Sqrt)
                rr = statp.tile([128, FH1], F32, tag="rr")
                nc.vector.reciprocal(rr[:], sqv[:])
                t1 = statp.tile([128, FH1], F32, tag="t1")
                for _ in range(2):
                    nc.vector.tensor_tensor(t1[:], rr[:], rr[:], AR.mult)
                    nc.vector.tensor_tensor(t1[:], t1[:], vpe[:], AR.mult)
                    nc.vector.tensor_scalar(t1[:], t1[:], -0.5, 1.5,
                                            AR.mult, AR.add)
                    nc.vector.tensor_tensor(rr[:], rr[:], t1[:], AR.mult)
                nc.vector.tensor_tensor(sTw[:, hs], rr[:], gsb[:, hs],
                                        AR.mult)
                # u = beta / sTw - mean
                rs = statp.tile([128, FH1], F32, tag="rs")
                nc.vector.reciprocal(rs[:], sTw[:, hs])
                uf = statp.tile([128, FH1], F32, tag="uf")
                nc.vector.tensor_tensor(uf[:], besb[:, hs], rs[:], AR.mult)
                nc.vector.tensor_tensor(uf[:], uf[:], mean[:], AR.subtract)
                nc.scalar.activation(u_bf[:, hs], uf[:], AF.Copy)

            bn_finalize(allst1, st_out1, slice(0, FH1))
            nc.sync.dma_start(dbg_d[:, 0:FH1], allst1[:, 0:FH1])
            nc.sync.dma_start(dbg_d[:, FCH:FCH + FH1], allst1[:, FH1:])

            # ================= phase 2 ===============================
            if V_P1ONLY:
                bn_finalize(allst2, st_out2, slice(FH1, FCH))
                nc.sync.dma_start(dbg_d[:, FH1:FCH], allst2[:, 0:FH1])
                nc.sync.dma_start(dbg_d[:, FCH + FH1:], allst2[:, FH1:])
                outsb0 = accp.tile([1, 1], F32)
                nc.vector.tensor_copy(outsb0[:], sTw[0:1, 0:1])
                nc.sync.dma_start(lp_d[:], outsb0[:])
            # l2w chunk 3 is the bias row: partition 0 holds cst8[k,:],
            # partitions 1..127 are zero; paired with hl chunk 3 whose
            # partition 0 is all-ones. This folds the per-(k,c) constant
            # into the second DoubleRow matmul - no separate bias matmul.
            l2w = l2wp.tile([128, NCB, 4, CS], F8)
            cstf = constp.tile([1, CS], F32)
            NSL = NF // 128  # 16 frame slots of 128
            ekall = statp.tile([128, NCB, NSL], F32)
            tgtall = statp.tile([128, NCB, 4], F32)

            with tc.tile_pool(name="ps2", bufs=4,
                              space=bass.MemorySpace.PSUM) as ps2:
                for k in range(NCB):
                    nc.vector.memset(l2w[:, k, 3, :], 0.0)
                def prep_k(k):
                    # fold BN scale into l2 weights; build cst row
                    l2r = l2rp.tile([128, 3, CS], F8, tag="l2r")
                    nc.sync.dma_start(l2r[:], l2t_d[k][:])
                    for c in range(3):
                        f = 3 * k + c
                        nc.scalar.activation(l2w[:, k, c, :], l2r[:, c, :],
                                             AF.Copy, scale=sTw[:, f:f + 1])
                    pcb = ps2.tile([128, 4, CS], F32, tag="p2")
                    pc = pcb[0:1, 0, :]
                    for c in range(3):
                        f = 3 * k + c
                        nc.tensor.matmul(pc, u_bf[:, f:f + 1],
                                         l2w[:, k, c, :],
                                         start=(c == 0), stop=(c == 2),
                                         skip_group_check=True)
                    nc.vector.tensor_tensor(cstf[:], pc,
                                            b2sb[0:1, k, :], AR.add)
                    nc.scalar.activation(l2w[0:1, k, 3, :], cstf[:], AF.Copy)

                def slots_k(k):
                    ohqk = ohqp.tile([128, NSL, CS], F8, tag="ohqk")
                    nc.sync.dma_start(ohqk[:], ohq_d[k][:])
                    # prefetch all 4 j2 hidden blocks up front
                    hls = []
                    for j2 in range(4):
                        hl = hlp.tile([128, 4, 512], F8, tag="hl")
                        for c in range(3):
                            nc.sync.dma_start(
                                hl[:, c, :],
                                scratch[3 * k + c][:, j2 * 512:(j2 + 1) * 512])
                        # chunk 3 = ones row (partition 0) for the folded
                        # bias; pairs with the cst row in l2w chunk 3
                        nc.sync.dma_start(hl[:, 3, :], oneck_d[:])
                        hls.append(hl)
                    for j2 in range(4):
                        hl = hls[j2]
                        pt2 = ps2.tile([128, 4, CS], F32, tag="p2")
                        # weight-grouped order: 4x pair1 then 4x pair2.
                        # start=True only on the first 1KB region of each
                        # 2KB PSUM zero-region (bank): the start marks the
                        # whole bank pending-zero, so the sibling region's
                        # first write (start=False) still zero-initializes;
                        # a second start=True would wipe the sibling's sum.
                        for q in range(4):
                            nc.tensor.matmul(
                                pt2[:, q, :],
                                hl[:, 0:2, q * 128:(q + 1) * 128],
                                l2w[:, k, 0:2, :],
                                start=(q % 2 == 0), stop=False,
                                perf_mode=DR, skip_group_check=True)
                        for q in range(4):
                            nc.tensor.matmul(
                                pt2[:, q, :],
                                hl[:, 2:4, q * 128:(q + 1) * 128],
                                l2w[:, k, 2:4, :],
                                start=False, stop=True,
                                perf_mode=DR, skip_group_check=True)
                        for q in range(4):
                            sl = j2 * 4 + q
                            ex = expp.tile([128, CS], BF16, tag="ex")
                            nc.scalar.activation(
                                ex[:], pt2[:, q, :], AF.Exp,
                                bias=0.0, scale=1.0 / L2S,
                                accum_out=ekall[:, k, sl:sl + 1])
                        # ohq holds 1/8 (not 1.0) -> descales the x8
                        # weight prescale in the target dot for free
                        dot = dotp.tile([128, 4, CS], F32, tag="dot")
                        nc.vector.tensor_tensor(
                            dot[:], pt2[:],
                            ohqk[:, 4 * j2:4 * j2 + 4, :], AR.mult)
                        nc.vector.tensor_reduce(
                            tgtall[:, k, j2:j2 + 1], dot[:],
                            mybir.AxisListType.XY, AR.add)

                if not V_P1ONLY:
                    for k in range(NCB // 2):
                        prep_k(k)
                    slots_k(0)
                    slots_k(1)
                    bn_finalize(allst2, st_out2, slice(FH1, FCH))
                    nc.sync.dma_start(dbg_d[:, FH1:FCH], allst2[:, 0:FH1])
                    nc.sync.dma_start(dbg_d[:, FCH + FH1:], allst2[:, FH1:])
                    for k in range(NCB // 2, NCB):
                        prep_k(k)
                    for k in range(2, NCB):
                        slots_k(k)

                if not V_P1ONLY:
                    # sum(tgt - lse) = sum(tgt) - sum(ln(esum)); one Ln
                    # instruction for all 256 (k, slot) esums
                    lseall = statp.tile([128, NCB, NSL], F32)
                    nc.scalar.activation(lseall[:], ekall[:], AF.Ln)
                    lsesum = accp.tile([128, 1], F32)
                    nc.vector.tensor_reduce(lsesum[:], lseall[:],
                                            mybir.AxisListType.XY, AR.add)
                    tgtsum = accp.tile([128, 1], F32)
                    nc.vector.tensor_reduce(tgtsum[:], tgtall[:],
                                            mybir.AxisListType.XY, AR.add)
                    total = accp.tile([128, 1], F32)
                    nc.vector.tensor_tensor(total[:], tgtsum[:], lsesum[:],
                                            AR.subtract)
                    ptFb = ps2.tile([128, 4, CS], F32, tag="p2")
                    ptF = ptFb[0:1, 0, 0:1]
                    nc.tensor.matmul(ptF, onesF[:], total[:], start=True,
                                     stop=True)
                    outsb = accp.tile([1, 1], F32)
                    nc.vector.tensor_copy(outsb[:], ptF)
                    nc.sync.dma_start(lp_d[:], outsb[:])

    nc.compile()
    return nc


def _prep_inputs(predictor, codebook_indexes, W1, b1, linear_self,
                 bn_gamma, bn_beta, linear2, bias2):
    f8 = ml_dtypes.float8_e4m3
    one8 = np.array(1.0, dtype=f8).view(np.uint8)
    ci = np.asarray(codebook_indexes).astype(np.int32)
    pred = np.asarray(predictor, dtype=np.float32)

    # replicated weights
    # W1 tiled: [f, kr, kc, m] = 32*W1[f*128+m, kc*128+kr]
    w1t = np.ascontiguousarray(
        (W1 * W1S).reshape(FCH, 128, KC, 128).transpose(0, 3, 2, 1)).astype(f8)
    # masked-by-construction LS: [f', cr, slot=k*2+cc, m] =
    #   32*linear_self[f'*128+m, k*256+cc*128+cr]
    ls = np.asarray(linear_self, dtype=np.float32) * W1S
    ls4 = ls.reshape(FCH - SELF_F0, 128, NCB - 1, 2, 128)  # [f', m, k, cc, cr]
    lst = np.zeros((FCH - SELF_F0, 128, MAXSLOT, 128), dtype=f8)
    lst_full = ls4.transpose(0, 4, 2, 3, 1).reshape(FCH - SELF_F0, 128,
                                                    MAXSLOT, 128)
    for fp in range(FCH - SELF_F0):
        nslot = ((fp // 3) + 1) * 2
        lst[fp, :, 0:nslot, :] = lst_full[fp, :, 0:nslot, :].astype(f8)
    # l2 tiled: [k, hr, hc, c] = 8*linear2[k, c, hc*128+hr]
    l2t = np.ascontiguousarray(
        (np.asarray(linear2, dtype=np.float32) * L2S)
        .reshape(NCB, CS, 3, 128).transpose(0, 3, 2, 1)).astype(f8)
    b2t = (np.asarray(bias2, dtype=np.float32) * L2S).reshape(1, NCB, CS)
    b1t = np.ascontiguousarray(
        np.asarray(b1, dtype=np.float32).reshape(FCH, 128).T)
    gt = np.ascontiguousarray(
        np.asarray(bn_gamma, dtype=np.float32).reshape(FCH, 128).T)
    bet = np.ascontiguousarray(
        np.asarray(bn_beta, dtype=np.float32).reshape(FCH, 128).T)
    onesF = np.ones((128, 1), dtype=np.float32)
    oneck = np.zeros((128, 512), dtype=f8)
    oneck[0, :] = 1.0

    rep = dict(w1t=w1t, lst=lst, l2t=l2t, b2t=b2t, b1t=b1t, gt=gt, bet=bet,
               onesF=onesF, oneck=oneck)

    NSL = NF // 128
    nn = np.arange(NF)
    kk = np.broadcast_to(np.arange(NCB), (NF, NCB))
    nn2 = np.broadcast_to(nn[:, None], (NF, NCB))
    in_maps = []
    for cidx in range(NCORES):
        sl = slice(cidx * NF, (cidx + 1) * NF)
        # predT tiled: [kr, kc, n] = pred[n, kc*128+kr]
        predT = np.ascontiguousarray(
            pred[sl].reshape(NF, KC, 128).transpose(2, 1, 0)).astype(f8)
        cil = ci[sl]                                   # (NF, NCB)
        valid = (cil >= 0) & (cil < CS)
        cc = np.clip(cil, 0, CS - 1)
        # ohT[r, k, hi, n] = 1 iff idx[n,k] == hi*128+r
        ohT = np.zeros((128, NCB, 2, NF), dtype=np.uint8)
        ohT[(cc % 128)[valid], kk[valid], (cc // 128)[valid],
            nn2[valid]] = one8
        # ohq[k, p, slot=n//128, c] = 1/8 iff idx[n,k] == c  (fp8);
        # the 1/8 undoes the x8 linear2 prescale in the target dot
        eighth = np.array(0.125, dtype=f8).view(np.uint8)
        ohq = np.zeros((NCB, 128, NSL, CS), dtype=np.uint8)
        ohq[kk[valid], (nn2 % 128)[valid], (nn2 // 128)[valid],
            cc[valid]] = eighth
        m = dict(predT=predT, ohT=ohT.view(f8), ohq=ohq.view(f8))
        m.update(rep)
        in_maps.append(m)
    return in_maps


def kernel(predictor, codebook_indexes, W1, b1, linear_self,
           bn_gamma, bn_beta, linear2, bias2):
    if "nc" not in _cached:
        _cached["nc"] = _build_program()
    nc = _cached["nc"]
    in_maps = _prep_inputs(predictor, codebook_indexes, W1, b1, linear_self,
                           bn_gamma, bn_beta, linear2, bias2)
    res = run_bass_kernel_spmd(nc, in_maps, list(range(NCORES)))
    _cached["last_results"] = res
    tot_logprob = np.float32(sum(float(r["lp"][0, 0]) for r in res.results))
    ci = np.asarray(codebook_indexes)
    tot_count = np.int32((ci[:, 0] >= 0).sum())
    return tot_logprob, tot_count


# revision 53
# speedup vs baseline: 1.0240x; 1.0043x over previous
"""Trainium2 Bass kernel for nn_JointCodebookPredictor.

Strategy: data-parallel over frames (N=16384 -> 2048/core on 8 cores).
Activations kept TRANSPOSED [feature, frame]; matmul contraction dims on
partitions. All big matmuls run fp8(e4m3) with perf_mode=DoubleRow
(2 contraction chunks per pass). Weights are pre-scaled on the host
(W1,LS x32; l2 x8) to dodge fp8 denormals; the scales are undone in the
ReLU evict (scale=1/32) and the exp/target ops (scale=1/8).

Phase 1 (single pass over the 48 feature chunks, full 2048 frames per
PSUM accumulation): hiddenT = relu(W1x32 @ predT + 32*selfpredT)/32+b1,
one-hot operands for the self-regression come pre-built from the host.
BN sum/sumsq accumulate via activation accum_out. hiddenT spilled to
DRAM scratch in fp8. BN stats all-reduce is split in two halves; the
first is issued mid-phase-1 so it overlaps compute.

Phase 2: BN is folded into the small linear2 weights (l2w = s*8*l2) and
a per-(k,c) constant (cst = u @ l2w + 8*b2, u = beta/s - mean), so the
spilled hidden feeds the logits matmul directly with no per-element BN.
logits psum = 8*logits; exp(x/8) with accum gives the softmax denoms;
a one-hot (1/8-valued) dot gives the target-logit sums; a single Ln over
all 256 (k,slot) denominators at the end avoids activation-table churn.
sum(tgt) - sum(lse) -> one scalar per core; the host sums 8 scalars.
"""

import os
import sys

sys.path.insert(0, "/opt/trn_rl_repo")

V_P1ONLY = os.environ.get("V_P1ONLY", "0") == "1"  # debug: skip phase 2

import numpy as np
import ml_dtypes

from concourse import bass, tile, bacc
import concourse.mybir as mybir
from concourse.bass_utils import run_bass_kernel_spmd

BF16 = mybir.dt.bfloat16
F32 = mybir.dt.float32
F8 = mybir.dt.float8e4

NCB = 16
CS = 256
HD = 384
PD = 2048
N_TOT = 16384
NCORES = 8
NF = N_TOT // NCORES          # 2048 frames per core
FCH = (NCB * HD) // 128       # 48 feature chunks of 128
KC = PD // 128                # 16 contraction chunks for mm1
BN_EPS = 1e-5
SELF_F0 = HD // 128           # first self feature chunk (=3)
MAXSLOT = (NCB - 1) * 2       # 30 slots in padded LS layout
FH1 = FCH // 2                # stats half boundary (f 0..23 = codebooks 0..7)
W1S = 32.0                    # host pre-scale on W1/LS
L2S = 8.0                     # host pre-scale on linear2/bias2

_cached = {}


def _build_program():
    nc = bacc.Bacc("TRN2", target_bir_lowering=False, debug=False,
                   num_devices=NCORES)

    # ---- DRAM I/O -------------------------------------------------------
    predT_d = nc.dram_tensor("predT", [128, KC, NF], F8, kind="ExternalInput")
    w1t_d = nc.dram_tensor("w1t", [FCH, 128, KC, 128], F8, kind="ExternalInput")
    lst_d = nc.dram_tensor("lst", [FCH - SELF_F0, 128, MAXSLOT, 128], F8,
                           kind="ExternalInput")
    ohT_d = nc.dram_tensor("ohT", [128, NCB, 2, NF], F8, kind="ExternalInput")
    ohq_d = nc.dram_tensor("ohq", [NCB, 128, NF // 128, CS], F8,
                           kind="ExternalInput")
    l2t_d = nc.dram_tensor("l2t", [NCB, 128, 3, CS], F8, kind="ExternalInput")
    b2t_d = nc.dram_tensor("b2t", [1, NCB, CS], F32, kind="ExternalInput")
    b1_d = nc.dram_tensor("b1t", [128, FCH], F32, kind="ExternalInput")
    g_d = nc.dram_tensor("gt", [128, FCH], F32, kind="ExternalInput")
    be_d = nc.dram_tensor("bet", [128, FCH], F32, kind="ExternalInput")
    ones_d = nc.dram_tensor("onesF", [128, 1], F32, kind="ExternalInput")
    oneck_d = nc.dram_tensor("oneck", [128, 512], F8, kind="ExternalInput")

    lp_d = nc.dram_tensor("lp", [1, 1], F32, kind="ExternalOutput")
    dbg_d = nc.dram_tensor("dbg", [128, 2 * FCH], F32, kind="ExternalOutput")

    AR = mybir.AluOpType
    AF = mybir.ActivationFunctionType
    DR = mybir.MatmulPerfMode.DoubleRow
    from contextlib import ExitStack

    with tile.TileContext(nc) as tc:
        with ExitStack() as ctx:
            ep = ctx.enter_context
            constp = ep(tc.tile_pool(name="const", bufs=1))
            predp = ep(tc.tile_pool(name="pred", bufs=1))
            ohp = ep(tc.tile_pool(name="oh", bufs=1))
            w1p = ep(tc.tile_pool(name="w1", bufs=2))
            lsp = ep(tc.tile_pool(name="ls", bufs=2))
            htp = ep(tc.tile_pool(name="ht", bufs=3))
            sqp = ep(tc.tile_pool(name="sq", bufs=2))
            statp = ep(tc.tile_pool(name="stat", bufs=1))
            l2wp = ep(tc.tile_pool(name="l2w", bufs=1))
            l2rp = ep(tc.tile_pool(name="l2r", bufs=3))
            ohqp = ep(tc.tile_pool(name="ohq", bufs=3))
            hlp = ep(tc.tile_pool(name="hload", bufs=6))
            expp = ep(tc.tile_pool(name="expp", bufs=6))
            dotp = ep(tc.tile_pool(name="dotp", bufs=3))
            accp = ep(tc.tile_pool(name="acc", bufs=1))
            dramp = ep(tc.tile_pool(name="dram", bufs=1, space="DRAM"))

            # ---- resident constants ---------------------------------
            b1sb = constp.tile([128, FCH], F32)
            nc.sync.dma_start(b1sb[:], b1_d[:])
            gsb = constp.tile([128, FCH], F32)
            nc.sync.dma_start(gsb[:], g_d[:])
            besb = constp.tile([128, FCH], F32)
            nc.sync.dma_start(besb[:], be_d[:])
            b2sb = constp.tile([1, NCB, CS], F32)
            nc.sync.dma_start(b2sb[:], b2t_d[:])
            onesF = constp.tile([128, 1], F32)
            nc.sync.dma_start(onesF[:], ones_d[:])

            # predT split per contraction-pair so the first matmul only
            # waits for 1/8 of the 4MB; first W1 slice loads before the
            # predT bulk; ohT loads are issued inside the f-loop (ohT[k]
            # first used at f=3+3k)
            predT = predp.tile([128, KC, NF], F8)
            w1sl0 = w1p.tile([128, KC, 128], F8, tag="w1sl")
            nc.sync.dma_start(predT[:, 0:2, :], predT_d[:, 0:2, :])
            nc.sync.dma_start(w1sl0[:], w1t_d[0][:])
            for c in range(1, KC // 2):
                nc.sync.dma_start(predT[:, 2 * c:2 * c + 2, :],
                                  predT_d[:, 2 * c:2 * c + 2, :])
            ohT = ohp.tile([128, NCB, 2, NF], F8)

            scratch = dramp.tile([FCH, 128, NF], F8)
            st_in1 = dramp.tile([128, 2 * FH1], F32)
            st_out1 = dramp.tile([128, 2 * FH1], F32)
            st_in2 = dramp.tile([128, 2 * FH1], F32)
            st_out2 = dramp.tile([128, 2 * FH1], F32)

            ssum = statp.tile([128, FCH], F32)
            ssq = statp.tile([128, FCH], F32)
            pack1 = statp.tile([128, 2 * FH1], F32)
            pack2 = statp.tile([128, 2 * FH1], F32)

            # ================= phase 1 ===============================
            with tc.tile_pool(name="ps1", bufs=2,
                              space=bass.MemorySpace.PSUM) as ps1:
                for f in range(FCH):
                    if f == 0:
                        w1sl = w1sl0
                    else:
                        w1sl = w1p.tile([128, KC, 128], F8, tag="w1sl")
                        nc.sync.dma_start(w1sl[:], w1t_d[f][:])
                    if f % 3 == 0 and f // 3 < NCB - 1:
                        kk = f // 3
                        nc.sync.dma_start(ohT[:, kk, :, :],
                                          ohT_d[:, kk, :, :])
                    i_blk = -1
                    if f >= SELF_F0:
                        i_blk = (f - SELF_F0) // 3
                        nslot = (i_blk + 1) * 2
                        lssl = lsp.tile([128, MAXSLOT, 128], F8, tag="lssl")
                        nc.sync.dma_start(lssl[:, 0:nslot, :],
                                          lst_d[f - SELF_F0][:, 0:nslot, :])
                    pt = ps1.tile([128, NF], F32, tag="p1")
                    for c in range(KC // 2):
                        for h in range(4):
                            hs = slice(h * 512, (h + 1) * 512)
                            last = (c == KC // 2 - 1) and i_blk < 0
                            nc.tensor.matmul(
                                pt[:, hs],
                                w1sl[:, 2 * c:2 * c + 2, :],
                                predT[:, 2 * c:2 * c + 2, hs],
                                start=(c == 0), stop=last,
                                perf_mode=DR, skip_group_check=True)
                    if i_blk >= 0:
                        for k in range(i_blk + 1):
                            for h in range(4):
                                hs = slice(h * 512, (h + 1) * 512)
                                nc.tensor.matmul(
                                    pt[:, hs],
                                    lssl[:, 2 * k:2 * k + 2, :],
                                    ohT[:, k, :, hs],
                                    start=False, stop=(k == i_blk),
                                    perf_mode=DR, skip_group_check=True)
                    # evict: relu(x/32 + b1), accumulate BN sum / sumsq
                    ht = htp.tile([128, NF], F8, tag="ht")
                    nc.scalar.activation(
                        ht[:], pt[:], AF.Relu,
                        bias=b1sb[:, f:f + 1], scale=1.0 / W1S,
                        accum_out=ssum[:, f:f + 1])
                    sq = sqp.tile([128, NF], BF16, tag="sq")
                    nc.vector.tensor_tensor(sq[:], ht[:], ht[:], AR.mult)
                    nc.vector.tensor_reduce(ssq[:, f:f + 1], sq[:],
                                            mybir.AxisListType.X, AR.add)
                    nc.sync.dma_start(scratch[f][:], ht[:])

                    if f == FH1 - 1:
                        # first-half stats allreduce, overlapped with
                        # the remaining phase-1 compute
                        nc.vector.tensor_copy(pack1[:, 0:FH1],
                                              ssum[:, 0:FH1])
                        nc.vector.tensor_copy(pack1[:, FH1:], ssq[:, 0:FH1])
                        nc.sync.dma_start(st_in1[:], pack1[:])
                        nc.gpsimd.collective_compute(
                            "AllReduce", AR.add,
                            replica_groups=[list(range(NCORES))],
                            ins=[st_in1.opt()], outs=[st_out1.opt()])


                nc.vector.tensor_copy(pack2[:, 0:FH1], ssum[:, FH1:])
                nc.vector.tensor_copy(pack2[:, FH1:], ssq[:, FH1:])
                nc.sync.dma_start(st_in2[:], pack2[:])
                nc.gpsimd.collective_compute(
                    "AllReduce", AR.add,
                    replica_groups=[list(range(NCORES))],
                    ins=[st_in2.opt()], outs=[st_out2.opt()])

            # ============ BN finalize (per half) =====================
            allst1 = statp.tile([128, 2 * FH1], F32)
            allst2 = statp.tile([128, 2 * FH1], F32)
            sTw = statp.tile([128, FCH], F32)    # gamma / std
            u_bf = statp.tile([128, FCH], F8)  # beta/sTw - mean

            def bn_finalize(allst, st_out, hs):
                nc.sync.dma_start(allst[:], st_out[:])
                mean = statp.tile([128, FH1], F32, tag="mean")
                nc.vector.tensor_scalar(mean[:], allst[:, 0:FH1],
                                        1.0 / N_TOT, None, AR.mult)
                vpe = statp.tile([128, FH1], F32, tag="vpe")
                nc.vector.tensor_scalar(vpe[:], allst[:, FH1:],
                                        1.0 / N_TOT, None, AR.mult)
                m2 = statp.tile([128, FH1], F32, tag="m2")
                nc.vector.tensor_tensor(m2[:], mean[:], mean[:], AR.mult)
                nc.vector.tensor_tensor(vpe[:], vpe[:], m2[:], AR.subtract)
                nc.vector.tensor_scalar(vpe[:], vpe[:], BN_EPS, None, AR.add)
                sqv = statp.tile([128, FH1], F32, tag="sqv")
                nc.scalar.activation(sqv[:], vpe[:], AF.Sqrt)
                rr = statp.tile([128, FH1], F32, tag="rr")
                nc.vector.reciprocal(rr[:], sqv[:])
                t1 = statp.tile([128, FH1], F32, tag="t1")
                for _ in range(2):
                    nc.vector.tensor_tensor(t1[:], rr[:], rr[:], AR.mult)
                    nc.vector.tensor_tensor(t1[:], t1[:], vpe[:], AR.mult)
                    nc.vector.tensor_scalar(t1[:], t1[:], -0.5, 1.5,
                                            AR.mult, AR.add)
                    nc.vector.tensor_tensor(rr[:], rr[:], t1[:], AR.mult)
                nc.vector.tensor_tensor(sTw[:, hs], rr[:], gsb[:, hs],
                                        AR.mult)
                # u = beta / sTw - mean
                rs = statp.tile([128, FH1], F32, tag="rs")
                nc.vector.reciprocal(rs[:], sTw[:, hs])
                uf = statp.tile([128, FH1], F32, tag="uf")
                nc.vector.tensor_tensor(uf[:], besb[:, hs], rs[:], AR.mult)
                nc.vector.tensor_tensor(uf[:], uf[:], mean[:], AR.subtract)
                nc.scalar.activation(u_bf[:, hs], uf[:], AF.Copy)

            bn_finalize(allst1, st_out1, slice(0, FH1))
            nc.sync.dma_start(dbg_d[:, 0:FH1], allst1[:, 0:FH1])
            nc.sync.dma_start(dbg_d[:, FCH:FCH + FH1], allst1[:, FH1:])

            # ================= phase 2 ===============================
            if V_P1ONLY:
                bn_finalize(allst2, st_out2, slice(FH1, FCH))
                nc.sync.dma_start(dbg_d[:, FH1:FCH], allst2[:, 0:FH1])
                nc.sync.dma_start(dbg_d[:, FCH + FH1:], allst2[:, FH1:])
                outsb0 = accp.tile([1, 1], F32)
                nc.vector.tensor_copy(outsb0[:], sTw[0:1, 0:1])
                nc.sync.dma_start(lp_d[:], outsb0[:])
            # l2w chunk 3 is the bias row: partition 0 holds cst8[k,:],
            # partitions 1..127 are zero; paired with hl chunk 3 whose
            # partition 0 is all-ones. This folds the per-(k,c) constant
            # into the second DoubleRow matmul - no separate bias matmul.
            l2w = l2wp.tile([128, NCB, 4, CS], F8)
            cstf = constp.tile([1, CS], F32)
            NSL = NF // 128  # 16 frame slots of 128
            ekall = statp.tile([128, NCB, NSL], F32)
            tgtall = statp.tile([128, NCB, 4], F32)

            with tc.tile_pool(name="ps2", bufs=4,
                              space=bass.MemorySpace.PSUM) as ps2:
                for k in range(NCB):
                    nc.vector.memset(l2w[:, k, 3, :], 0.0)
                def prep_k(k):
                    # fold BN scale into l2 weights; build cst row
                    l2r = l2rp.tile([128, 3, CS], F8, tag="l2r")
                    nc.sync.dma_start(l2r[:], l2t_d[k][:])
                    for c in range(3):
                        f = 3 * k + c
                        nc.scalar.activation(l2w[:, k, c, :], l2r[:, c, :],
                                             AF.Copy, scale=sTw[:, f:f + 1])
                    pcb = ps2.tile([128, 4, CS], F32, tag="p2")
                    pc = pcb[0:1, 0, :]
                    for c in range(3):
                        f = 3 * k + c
                        nc.tensor.matmul(pc, u_bf[:, f:f + 1],
                                         l2w[:, k, c, :],
                                         start=(c == 0), stop=(c == 2),
                                         skip_group_check=True)
                    nc.vector.tensor_tensor(cstf[:], pc,
                                            b2sb[0:1, k, :], AR.add)
                    nc.scalar.activation(l2w[0:1, k, 3, :], cstf[:], AF.Copy)

                def slots_k(k):
                    ohqk = ohqp.tile([128, NSL, CS], F8, tag="ohqk")
                    nc.sync.dma_start(ohqk[:], ohq_d[k][:])
                    # prefetch all 4 j2 hidden blocks up front
                    hls = []
                    for j2 in range(4):
                        hl = hlp.tile([128, 4, 512], F8, tag="hl")
                        for c in range(3):
                            nc.sync.dma_start(
                                hl[:, c, :],
                                scratch[3 * k + c][:, j2 * 512:(j2 + 1) * 512])
                        # chunk 3 = ones row (partition 0) for the folded
                        # bias; pairs with the cst row in l2w chunk 3
                        nc.sync.dma_start(hl[:, 3, :], oneck_d[:])
                        hls.append(hl)
                    for j2 in range(4):
                        hl = hls[j2]
                        pt2 = ps2.tile([128, 4, CS], F32, tag="p2")
                        # weight-grouped order: 4x pair1 then 4x pair2.
                        # start=True only on the first 1KB region of each
                        # 2KB PSUM zero-region (bank): the start marks the
                        # whole bank pending-zero, so the sibling region's
                        # first write (start=False) still zero-initializes;
                        # a second start=True would wipe the sibling's sum.
                        for q in range(4):
                            nc.tensor.matmul(
                                pt2[:, q, :],
                                hl[:, 0:2, q * 128:(q + 1) * 128],
                                l2w[:, k, 0:2, :],
                                start=(q % 2 == 0), stop=False,
                                perf_mode=DR, skip_group_check=True)
                        for q in range(4):
                            nc.tensor.matmul(
                                pt2[:, q, :],
                                hl[:, 2:4, q * 128:(q + 1) * 128],
                                l2w[:, k, 2:4, :],
                                start=False, stop=True,
                                perf_mode=DR, skip_group_check=True)
                        for q in range(4):
                            sl = j2 * 4 + q
                            ex = expp.tile([128, CS], BF16, tag="ex")
                            nc.scalar.activation(
                                ex[:], pt2[:, q, :], AF.Exp,
                                bias=0.0, scale=1.0 / L2S,
                                accum_out=ekall[:, k, sl:sl + 1])
                        # ohq holds 1/8 (not 1.0) -> descales the x8
                        # weight prescale in the target dot for free
                        dot = dotp.tile([128, 4, CS], F32, tag="dot")
                        nc.vector.tensor_tensor(
                            dot[:], pt2[:],
                            ohqk[:, 4 * j2:4 * j2 + 4, :], AR.mult)
                        nc.vector.tensor_reduce(
                            tgtall[:, k, j2:j2 + 1], dot[:],
                            mybir.AxisListType.XY, AR.add)

                if not V_P1ONLY:
                    for k in range(NCB // 2):
                        prep_k(k)
                    for k in range(4):
                        slots_k(k)
                    bn_finalize(allst2, st_out2, slice(FH1, FCH))
                    nc.sync.dma_start(dbg_d[:, FH1:FCH], allst2[:, 0:FH1])
                    nc.sync.dma_start(dbg_d[:, FCH + FH1:], allst2[:, FH1:])
                    for k in range(NCB // 2, NCB):
                        prep_k(k)
                    for k in range(4, NCB):
                        slots_k(k)

                if not V_P1ONLY:
                    # sum(tgt - lse) = sum(tgt) - sum(ln(esum)); one Ln
                    # instruction for all 256 (k, slot) esums
                    lseall = statp.tile([128, NCB, NSL], F32)
                    nc.scalar.activation(lseall[:], ekall[:], AF.Ln)
                    lsesum = accp.tile([128, 1], F32)
                    nc.vector.tensor_reduce(lsesum[:], lseall[:],
                                            mybir.AxisListType.XY, AR.add)
                    tgtsum = accp.tile([128, 1], F32)
                    nc.vector.tensor_reduce(tgtsum[:], tgtall[:],
                                            mybir.AxisListType.XY, AR.add)
                    total = accp.tile([128, 1], F32)
                    nc.vector.tensor_tensor(total[:], tgtsum[:], lsesum[:],
                                            AR.subtract)
                    ptFb = ps2.tile([128, 4, CS], F32, tag="p2")
                    ptF = ptFb[0:1, 0, 0:1]
                    nc.tensor.matmul(ptF, onesF[:], total[:], start=True,
                                     stop=True)
                    outsb = accp.tile([1, 1], F32)
                    nc.vector.tensor_copy(outsb[:], ptF)
                    nc.sync.dma_start(lp_d[:], outsb[:])

    nc.compile()
    return nc


def _prep_inputs(predictor, codebook_indexes, W1, b1, linear_self,
                 bn_gamma, bn_beta, linear2, bias2):
    f8 = ml_dtypes.float8_e4m3
    one8 = np.array(1.0, dtype=f8).view(np.uint8)
    ci = np.asarray(codebook_indexes).astype(np.int32)
    pred = np.asarray(predictor, dtype=np.float32)

    # replicated weights
    # W1 tiled: [f, kr, kc, m] = 32*W1[f*128+m, kc*128+kr]
    w1t = np.ascontiguousarray(
        (W1 * W1S).reshape(FCH, 128, KC, 128).transpose(0, 3, 2, 1)).astype(f8)
    # masked-by-construction LS: [f', cr, slot=k*2+cc, m] =
    #   32*linear_self[f'*128+m, k*256+cc*128+cr]
    ls = np.asarray(linear_self, dtype=np.float32) * W1S
    ls4 = ls.reshape(FCH - SELF_F0, 128, NCB - 1, 2, 128)  # [f', m, k, cc, cr]
    lst = np.zeros((FCH - SELF_F0, 128, MAXSLOT, 128), dtype=f8)
    lst_full = ls4.transpose(0, 4, 2, 3, 1).reshape(FCH - SELF_F0, 128,
                                                    MAXSLOT, 128)
    for fp in range(FCH - SELF_F0):
        nslot = ((fp // 3) + 1) * 2
        lst[fp, :, 0:nslot, :] = lst_full[fp, :, 0:nslot, :].astype(f8)
    # l2 tiled: [k, hr, hc, c] = 8*linear2[k, c, hc*128+hr]
    l2t = np.ascontiguousarray(
        (np.asarray(linear2, dtype=np.float32) * L2S)
        .reshape(NCB, CS, 3, 128).transpose(0, 3, 2, 1)).astype(f8)
    b2t = (np.asarray(bias2, dtype=np.float32) * L2S).reshape(1, NCB, CS)
    b1t = np.ascontiguousarray(
        np.asarray(b1, dtype=np.float32).reshape(FCH, 128).T)
    gt = np.ascontiguousarray(
        np.asarray(bn_gamma, dtype=np.float32).reshape(FCH, 128).T)
    bet = np.ascontiguousarray(
        np.asarray(bn_beta, dtype=np.float32).reshape(FCH, 128).T)
    onesF = np.ones((128, 1), dtype=np.float32)
    oneck = np.zeros((128, 512), dtype=f8)
    oneck[0, :] = 1.0

    rep = dict(w1t=w1t, lst=lst, l2t=l2t, b2t=b2t, b1t=b1t, gt=gt, bet=bet,
               onesF=onesF, oneck=oneck)

    NSL = NF // 128
    nn = np.arange(NF)
    kk = np.broadcast_to(np.arange(NCB), (NF, NCB))
    nn2 = np.broadcast_to(nn[:, None], (NF, NCB))
    in_maps = []
    for cidx in range(NCORES):
        sl = slice(cidx * NF, (cidx + 1) * NF)
        # predT tiled: [kr, kc, n] = pred[n, kc*128+kr]
        predT = np.ascontiguousarray(
            pred[sl].reshape(NF, KC, 128).transpose(2, 1, 0)).astype(f8)
        cil = ci[sl]                                   # (NF, NCB)
        valid = (cil >= 0) & (cil < CS)
        cc = np.clip(cil, 0, CS - 1)
        # ohT[r, k, hi, n] = 1 iff idx[n,k] == hi*128+r
        ohT = np.zeros((128, NCB, 2, NF), dtype=np.uint8)
        ohT[(cc % 128)[valid], kk[valid], (cc // 128)[valid],
            nn2[valid]] = one8
        # ohq[k, p, slot=n//128, c] = 1/8 iff idx[n,k] == c  (fp8);
        # the 1/8 undoes the x8 linear2 prescale in the target dot
        eighth = np.array(0.125, dtype=f8).view(np.uint8)
        ohq = np.zeros((NCB, 128, NSL, CS), dtype=np.uint8)
        ohq[kk[valid], (nn2 % 128)[valid], (nn2 // 128)[valid],
            cc[valid]] = eighth
        m = dict(predT=predT, ohT=ohT.view(f8), ohq=ohq.view(f8))
        m.update(rep)
        in_maps.append(m)
    return in_maps


def kernel(predictor, codebook_indexes, W1, b1, linear_self,
           bn_gamma, bn_beta, linear2, bias2):
    if "nc" not in _cached:
        _cached["nc"] = _build_program()
    nc = _cached["nc"]
    in_maps = _prep_inputs(predictor, codebook_indexes, W1, b1, linear_self,
                           bn_gamma, bn_beta, linear2, bias2)
    res = run_bass_kernel_spmd(nc, in_maps, list(range(NCORES)))
    _cached["last_results"] = res
    tot_logprob = np.float32(sum(float(r["lp"][0, 0]) for r in res.results))
    ci = np.asarray(codebook_indexes)
    tot_count = np.int32((ci[:, 0] >= 0).sum())
    return tot_logprob, tot_count
